# revision 17
# baseline (speedup 1.0000x reference)
"""Multi-head attention (B=4, S=2048, D=1024, H=16, causal) on 8 TRN2 NeuronCores.

Sharding: core c -> batch b = c // 2, head-group g = c % 2 (8 heads, 512 dims).
Each core computes its heads' projections + full SxS causal attention + its
partial output projection; the host sums the two head-group partials per batch
and adds the output bias.
"""

import os
import numpy as np

B, S, D = 4, 2048, 1024
H, DK = 16, 64
N_CORES = 8
DH = 512          # head dims per core (8 heads x 64)
P = 128           # partitions
KT = D // P       # 8 k-slabs
NPAIR = 4         # head pairs per core
SC = S // 512     # 4 s-chunks of 512
ST = S // P       # 16 s-tiles of 128
VW = 8 * (DK + 1)  # 520: v storage row width per s-tile (8 heads x (64 V + 1 ones))

_CACHE = {}
LAST_EXEC_NS = None


def _build():
    from contextlib import ExitStack

    import concourse.bass as bass
    import concourse.tile as tile
    from concourse import bacc, mybir
    from concourse.masks import make_identity

    f32 = mybir.dt.float32
    f32r = mybir.dt.float32r
    bf16 = mybir.dt.bfloat16
    AF = mybir.ActivationFunctionType
    OP = mybir.AluOpType

    nc = bacc.Bacc("TRN2", target_bir_lowering=False, debug=False,
                   num_devices=N_CORES)

    xb = nc.dram_tensor("xb", [S, D], f32, kind="ExternalInput").ap()
    wq = nc.dram_tensor("wq", [DH, D], f32, kind="ExternalInput").ap()
    wk = nc.dram_tensor("wk", [DH, D], f32, kind="ExternalInput").ap()
    wv = nc.dram_tensor("wv", [DH, D], f32, kind="ExternalInput").ap()
    wo = nc.dram_tensor("wo", [D, DH], f32, kind="ExternalInput").ap()
    bq = nc.dram_tensor("bq", [DH], f32, kind="ExternalInput").ap()
    bk = nc.dram_tensor("bk", [DH], f32, kind="ExternalInput").ap()
    bv = nc.dram_tensor("bv", [DH], f32, kind="ExternalInput").ap()
    y = nc.dram_tensor("y", [S, D], f32, kind="ExternalOutput").ap()

    # bf16 staging in DRAM for the XBAR dma-transposes (weights only)
    wq_bf = nc.dram_tensor("wq_bf", [DH, D], bf16).ap()
    wk_bf = nc.dram_tensor("wk_bf", [DH, D], bf16).ap()
    wv_bf = nc.dram_tensor("wv_bf", [DH, D], bf16).ap()
    wo_bf = nc.dram_tensor("wo_bf", [D, DH], bf16).ap()

    with tile.TileContext(nc) as tc, ExitStack() as ctx:
        from contextlib import ExitStack as _ES

        persist = ctx.enter_context(tc.tile_pool(name="persist", bufs=1))
        sb_exp = ctx.enter_context(tc.tile_pool(name="sb_exp", bufs=4))
        sb_y = ctx.enter_context(tc.tile_pool(name="sb_y", bufs=3))
        sb_rab = ctx.enter_context(tc.tile_pool(name="sb_rab", bufs=2))
        sb_rsb = ctx.enter_context(tc.tile_pool(name="sb_rsb", bufs=2))
        proj_ctx = ctx.enter_context(_ES())
        ps_work = proj_ctx.enter_context(tc.tile_pool(name="ps_work", bufs=4, space="PSUM"))
        ps_tr = proj_ctx.enter_context(tc.tile_pool(name="ps_tr", bufs=4, space="PSUM"))
        sb_xn = proj_ctx.enter_context(tc.tile_pool(name="sb_xn", bufs=4))

        # persistent SBUF tensors
        xT = persist.tile([P, KT * S], bf16, tag="xT")            # k-slab k at cols [k*S, (k+1)*S)
        wqT = persist.tile([P, KT * DH], bf16, tag="wqT")         # [128k, 512dq] per slab
        wkT = persist.tile([P, KT * DH], bf16, tag="wkT")
        wvT = persist.tile([P, KT * DH], bf16, tag="wvT")
        woT = persist.tile([P, NPAIR * D], bf16, tag="woT")       # d-slab dt at cols [dt*D, ...)
        qT = persist.tile([P, NPAIR * S], bf16, tag="qT")         # pair p at cols [p*S, ...)
        kTt = persist.tile([P, NPAIR * S], bf16, tag="kTt")
        vS = persist.tile([P, ST * VW], bf16, tag="vS")           # s-tile jt at cols [jt*VW, ...)
        ctxT = persist.tile([P, NPAIR * S], bf16, tag="ctxT")
        mask2 = persist.tile([P, 4 * 1024], bf16, tag="mask2")    # 4 diagonal masks, doubled
        bq_sb = persist.tile([P, NPAIR], f32, tag="bq_sb")
        bk_sb = persist.tile([P, NPAIR], f32, tag="bk_sb")
        bv_sb = persist.tile([1, DH], f32, tag="bv_sb")
        bv_bc = persist.tile([P, DH], f32, tag="bv_bc")           # bv broadcast to 128 partitions
        ones_r = persist.tile([1, P], f32, tag="ones_r")          # ones row (partition 0)
        ones_f = persist.tile([P, 512], f32, tag="ones_f")

        # ---- staging ----
        # weights: cast to bf16 in DRAM, XBAR-transpose into SBUF (sync ring only:
        # concurrent transposes on the scalar HWDGE ring corrupt data via the xbar)
        nc.gpsimd.dma_start(out=wq_bf[:, :], in_=wq[:, :])
        nc.gpsimd.dma_start(out=wk_bf[:, :], in_=wk[:, :])
        nc.gpsimd.dma_start(out=wv_bf[:, :], in_=wv[:, :])
        nc.gpsimd.dma_start(out=wo_bf[:, :], in_=wo[:, :])
        for k in range(KT):
            cs = slice(k * P, (k + 1) * P)
            nc.sync.dma_start(out=wqT[:, k * DH:(k + 1) * DH], in_=wq_bf[:, cs], transpose=True)
            nc.sync.dma_start(out=wkT[:, k * DH:(k + 1) * DH], in_=wk_bf[:, cs], transpose=True)
            nc.sync.dma_start(out=wvT[:, k * DH:(k + 1) * DH], in_=wv_bf[:, cs], transpose=True)
        for dt in range(NPAIR):
            cs = slice(dt * P, (dt + 1) * P)
            nc.sync.dma_start(out=woT[:, dt * D:(dt + 1) * D], in_=wo_bf[:, cs], transpose=True)
        # x: cast-DMA straight to SBUF per s-tile, then PE-transpose 128x128 blocks
        # (PE is idle during staging; PSUM->SBUF copies go to the idle ACT engine)
        ident = persist.tile([P, P], bf16, tag="ident")
        make_identity(nc, ident[:])
        for st in range(ST):
            xn = sb_xn.tile([P, D], bf16, tag="xn")
            nc.gpsimd.dma_start(out=xn[:], in_=xb[st * P:(st + 1) * P, :])
            for k in range(KT):
                pt = ps_tr.tile([P, P], bf16, tag="ptr")
                nc.tensor.transpose(pt[:], xn[:, k * P:(k + 1) * P], ident[:])
                nc.scalar.copy(xT[:, k * S + st * P: k * S + (st + 1) * P], pt[:])

        # biases + constants
        for p in range(NPAIR):
            nc.sync.dma_start(out=bq_sb[:, p:p + 1], in_=bq[p * P:(p + 1) * P])
            nc.sync.dma_start(out=bk_sb[:, p:p + 1], in_=bk[p * P:(p + 1) * P])
        nc.sync.dma_start(out=bv_sb[0:1, :], in_=bv[:])
        nc.gpsimd.memset(ones_r[:], 1.0)
        nc.gpsimd.memset(ones_f[:], 1.0)
        nc.gpsimd.memset(vS[:], 1.0)

        # bv broadcast to all partitions (gpsimd)
        nc.gpsimd.partition_broadcast(bv_bc[:], bv_sb[0:1, :])

        # causal masks for the 4 diagonal offsets, stored doubled ([mask | mask])
        for di in range(4):
            d = di * 128
            for half in range(2):
                nc.gpsimd.affine_select(
                    out=mask2[:, di * 1024 + half * 512: di * 1024 + (half + 1) * 512],
                    in_=ones_f[:],
                    pattern=[[1, 512]],
                    compare_op=OP.is_ge,
                    fill=0.0,
                    base=-d,
                    channel_multiplier=-1,
                )

        # ---- projections ----
        # V natural [s, dv] + bias, written strided with a ones column per head
        for st in range(ST):
            vp = ps_work.tile([P, 512], f32, tag="work")
            for k in range(KT):
                nc.tensor.matmul(
                    vp[:],
                    xT[:, k * S + st * P: k * S + (st + 1) * P],
                    wvT[:, k * DH:(k + 1) * DH],
                    start=(k == 0), stop=(k == KT - 1))
            vdst = vS[:, st * VW:(st + 1) * VW].rearrange("p (h c) -> p h c", c=DK + 1)[:, :, 0:DK]
            nc.vector.tensor_tensor(
                vdst,
                vp[:].rearrange("p (h c) -> p h c", c=DK),
                bv_bc[:].rearrange("p (h c) -> p h c", c=DK),
                OP.add)

        # Q^T, K^T: [128 (pair dims), S] slabs; out = W_sliceT.T @ xT
        for name, wT, bias_sb, out_sb in (("q", wqT, bq_sb, qT), ("k", wkT, bk_sb, kTt)):
            for p in range(NPAIR):
                psums = [ps_work.tile([P, 512], f32, tag="work", name=f"proj_ps{_i}") for _i in range(SC)]
                for k in range(KT):
                    for sc in range(SC):
                        nc.tensor.matmul(
                            psums[sc][:],
                            wT[:, k * DH + p * P: k * DH + (p + 1) * P],
                            xT[:, k * S + sc * 512: k * S + (sc + 1) * 512],
                            start=(k == 0), stop=(k == KT - 1))
                for sc in range(SC):
                    nc.vector.tensor_scalar_add(
                        out_sb[:, p * S + sc * 512: p * S + (sc + 1) * 512],
                        psums[sc][:], bias_sb[:, p:p + 1])

        # ---- attention ----
        proj_ctx.close()
        attn_ctx = ctx.enter_context(_ES())
        ps_scores = attn_ctx.enter_context(tc.tile_pool(name="ps_scores", bufs=2, space="PSUM"))
        ps_acc = attn_ctx.enter_context(tc.tile_pool(name="ps_acc", bufs=2, space="PSUM"))
        for ic in range(SC):
            for p in range(NPAIR):
                accAB = ps_acc.tile([DK + 1, 1024], f32, tag="acc")
                accA = accAB[:, 0:512]
                accB = accAB[:, 512:1024]
                njt = 4 * ic + 4
                for jt in range(njt):
                    sps = ps_scores.tile([P, 1024], f32, tag="scores")
                    # scores^T for both heads of the pair, row-packed (K=64)
                    nc.tensor.matmul(
                        sps[:, 0:512],
                        kTt[0:DK, p * S + jt * P: p * S + (jt + 1) * P],
                        qT[0:DK, p * S + ic * 512: p * S + (ic + 1) * 512],
                        start=True, stop=True)
                    nc.tensor.matmul(
                        sps[:, 512:1024],
                        kTt[DK:P, p * S + jt * P: p * S + (jt + 1) * P],
                        qT[DK:P, p * S + ic * 512: p * S + (ic + 1) * 512],
                        start=True, stop=True)
                    diag = jt >= 4 * ic
                    d = (jt - 4 * ic) * P if diag else 0
                    ex = sb_exp.tile([P, 1024], bf16, tag="exp")
                    nc.scalar.activation(ex[:, d:1024], sps[:, d:1024], AF.Exp, scale=0.125)
                    if diag:  # causal mask on the partial diagonal tile
                        di = jt - 4 * ic
                        nc.vector.tensor_mul(
                            ex[:, d:1024], ex[:, d:1024],
                            mask2[:, di * 1024 + d:(di + 1) * 1024])
                    for hl, acc in ((0, accA), (1, accB)):
                        hv = 2 * p + hl
                        nc.tensor.matmul(
                            acc[:, d:512],
                            vS[:, jt * VW + hv * (DK + 1): jt * VW + (hv + 1) * (DK + 1)],
                            ex[:, hl * 512 + d:(hl + 1) * 512],
                            start=(jt == 0), stop=(jt == njt - 1))
                # normalize: r = 1/sumexp (fast approx), gpsimd-broadcast to all partitions
                sraw = sb_rab.tile([1, 1024], f32, tag="sraw")
                nc.vector.tensor_copy(sraw[0:1, :], accAB[DK:DK + 1, :])
                rab = sb_rab.tile([1, 1024], f32, tag="rab")
                nc.vector.reciprocal_approx_fast(rab[0:1, :], sraw[0:1, :])
                Rs = sb_rsb.tile([P, 1024], f32, tag="rsb")
                nc.gpsimd.partition_broadcast(Rs[:], rab[0:1, :])
                for hl, acc in ((0, accA), (1, accB)):
                    nc.vector.tensor_mul(
                        ctxT[hl * DK:(hl + 1) * DK, p * S + ic * 512: p * S + (ic + 1) * 512],
                        acc[0:DK, :],
                        Rs[hl * DK:(hl + 1) * DK, hl * 512:(hl + 1) * 512])

        # ---- output projection: y[s, m] partial ----
        attn_ctx.close()
        ps_oproj = ctx.enter_context(tc.tile_pool(name="ps_oproj", bufs=4, space="PSUM"))
        for st in range(ST):
            for mc in range(2):
                yp = ps_oproj.tile([P, 512], f32, tag="work")
                for dt in range(NPAIR):
                    nc.tensor.matmul(
                        yp[:],
                        ctxT[:, dt * S + st * P: dt * S + (st + 1) * P],
                        woT[:, dt * D + mc * 512: dt * D + (mc + 1) * 512],
                        start=(dt == 0), stop=(dt == NPAIR - 1))
                yt = sb_y.tile([P, 512], f32, tag="yout")
                nc.vector.tensor_copy(yt[:], yp[:])
                nc.sync.dma_start(
                    out=y[st * P:(st + 1) * P, mc * 512:(mc + 1) * 512], in_=yt[:])

    nc.compile()
    return nc


def _get_nc():
    if "nc" not in _CACHE:
        _CACHE["nc"] = _build()
    return _CACHE["nc"]


def kernel(x, mask, Wq, bq, Wk, bk, Wv, bv, Wo, bo, **_unused):
    global LAST_EXEC_NS
    from concourse.bass_utils import run_bass_kernel_spmd

    x = np.asarray(x, dtype=np.float32)
    Wq = np.asarray(Wq, dtype=np.float32)
    Wk = np.asarray(Wk, dtype=np.float32)
    Wv = np.asarray(Wv, dtype=np.float32)
    Wo = np.asarray(Wo, dtype=np.float32)
    bq = np.asarray(bq, dtype=np.float32)
    bk = np.asarray(bk, dtype=np.float32)
    bv = np.asarray(bv, dtype=np.float32)
    bo = np.asarray(bo, dtype=np.float32)

    nc = _get_nc()
    in_maps = []
    for c in range(N_CORES):
        b, g = c // 2, c % 2
        r = slice(g * DH, (g + 1) * DH)
        in_maps.append({
            "xb": np.ascontiguousarray(x[b]),
            "wq": np.ascontiguousarray(Wq[r]),
            "wk": np.ascontiguousarray(Wk[r]),
            "wv": np.ascontiguousarray(Wv[r]),
            "wo": np.ascontiguousarray(Wo[:, r]),
            "bq": np.ascontiguousarray(bq[r]),
            "bk": np.ascontiguousarray(bk[r]),
            "bv": np.ascontiguousarray(bv[r]),
        })

    res = run_bass_kernel_spmd(nc, in_maps, list(range(N_CORES)),
                               trace=bool(os.environ.get("BASS_TRACE")))
    LAST_EXEC_NS = res.exec_time_ns

    out = np.zeros((B, S, D), dtype=np.float32)
    for c in range(N_CORES):
        out[c // 2] += res.results[c]["y"]
    out += bo[None, None, :]
    return out


# revision 19
# speedup vs baseline: 1.0181x; 1.0181x over previous
"""Multi-head attention (B=4, S=2048, D=1024, H=16, causal) on 8 TRN2 NeuronCores.

Sharding: core c -> batch b = c // 2, head-group g = c % 2 (8 heads, 512 dims).
Each core computes its heads' projections + full SxS causal attention + its
partial output projection; the host sums the two head-group partials per batch
and adds the output bias.

Per-core pipeline (all matmuls bf16 with fp32 PSUM accumulate):
  - inputs cast fp32->bf16 during DMA (SWDGE), transposed on the PE with an
    identity matmul (XBAR dma-transpose is avoided: it is slow to issue and
    concurrent use of both HWDGE rings corrupts data)
  - Q^T/K^T per head-pair slabs [128, S]; V kept natural with a ones column
    per head (ones-augmented V makes attn@V also produce sumexp rows)
  - scores^T tiles [128 j, 512 i] computed with two K=64 row-packed matmuls
    (both heads of a pair concurrently in the PE array)
  - exp on ScalarE straight out of PSUM (no max subtraction: scores are
    bounded, verified |s| <= 9.5), causal masking only on diagonal tiles via
    precomputed affine_select masks, diagonal tiles only computed from their
    first in-range column
  - softmax normalization: 1/sumexp via one reciprocal_approx_fast, then
    gpsimd partition_broadcast, then one DVE multiply per head
  - output projection y = ctx^T.T @ Wo_slice^T interleaved per i-chunk
"""

import os
import numpy as np

B, S, D = 4, 2048, 1024
H, DK = 16, 64
N_CORES = 8
DH = 512          # head dims per core (8 heads x 64)
P = 128           # partitions
KT = D // P       # 8 k-slabs
NPAIR = 4         # head pairs per core
SC = S // 512     # 4 s-chunks of 512
ST = S // P       # 16 s-tiles of 128
VW = 8 * (DK + 1)  # 520: v storage row width per s-tile (8 heads x (64 V + 1 ones))

_CACHE = {}
LAST_EXEC_NS = None


def _build():
    from contextlib import ExitStack

    import concourse.bass as bass
    import concourse.tile as tile
    from concourse import bacc, mybir
    from concourse.masks import make_identity

    f32 = mybir.dt.float32
    bf16 = mybir.dt.bfloat16
    AF = mybir.ActivationFunctionType
    OP = mybir.AluOpType

    nc = bacc.Bacc("TRN2", target_bir_lowering=False, debug=False,
                   num_devices=N_CORES)

    xb = nc.dram_tensor("xb", [S, D], f32, kind="ExternalInput").ap()
    wq = nc.dram_tensor("wq", [DH, D], f32, kind="ExternalInput").ap()
    wk = nc.dram_tensor("wk", [DH, D], f32, kind="ExternalInput").ap()
    wv = nc.dram_tensor("wv", [DH, D], f32, kind="ExternalInput").ap()
    wo = nc.dram_tensor("wo", [D, DH], f32, kind="ExternalInput").ap()
    bq = nc.dram_tensor("bq", [DH], f32, kind="ExternalInput").ap()
    bk = nc.dram_tensor("bk", [DH], f32, kind="ExternalInput").ap()
    bv = nc.dram_tensor("bv", [DH], f32, kind="ExternalInput").ap()
    y = nc.dram_tensor("y", [S, D], f32, kind="ExternalOutput").ap()

    with tile.TileContext(nc) as tc, ExitStack() as ctx:
        from contextlib import ExitStack as _ES

        persist = ctx.enter_context(tc.tile_pool(name="persist", bufs=1))
        sb_exp = ctx.enter_context(tc.tile_pool(name="sb_exp", bufs=4))
        sb_y = ctx.enter_context(tc.tile_pool(name="sb_y", bufs=3))
        sb_rab = ctx.enter_context(tc.tile_pool(name="sb_rab", bufs=2))
        sb_rsb = ctx.enter_context(tc.tile_pool(name="sb_rsb", bufs=2))

        # persistent SBUF tensors
        xT = persist.tile([P, KT * S], bf16, tag="xT")            # k-slab k at cols [k*S, (k+1)*S)
        wqT = persist.tile([P, KT * DH], bf16, tag="wqT")         # [128k, 512dq] per slab
        wkT = persist.tile([P, KT * DH], bf16, tag="wkT")
        wvT = persist.tile([P, KT * DH], bf16, tag="wvT")
        woT = persist.tile([P, NPAIR * D], bf16, tag="woT")       # d-slab dt at cols [dt*D, ...)
        qT = persist.tile([P, NPAIR * S], bf16, tag="qT")         # pair p at cols [p*S, ...)
        kTt = persist.tile([P, NPAIR * S], bf16, tag="kTt")
        vS = persist.tile([P, ST * VW], bf16, tag="vS")           # s-tile jt at cols [jt*VW, ...)
        ctxT = persist.tile([P, NPAIR * S], bf16, tag="ctxT")
        mask2 = persist.tile([P, 4 * 1024], bf16, tag="mask2")    # 4 diagonal masks, doubled
        bq_sb = persist.tile([P, NPAIR], f32, tag="bq_sb")
        bk_sb = persist.tile([P, NPAIR], f32, tag="bk_sb")
        bv_sb = persist.tile([1, DH], f32, tag="bv_sb")
        bv_bc = persist.tile([P, DH], f32, tag="bv_bc")
        ones_f = persist.tile([P, 512], f32, tag="ones_f")
        ident = persist.tile([P, P], bf16, tag="ident")

        proj_ctx = ctx.enter_context(_ES())
        ps_work = proj_ctx.enter_context(tc.tile_pool(name="ps_work", bufs=4, space="PSUM"))
        ps_tr = proj_ctx.enter_context(tc.tile_pool(name="ps_tr", bufs=4, space="PSUM"))
        sb_nat = proj_ctx.enter_context(tc.tile_pool(name="sb_nat", bufs=6))

        make_identity(nc, ident[:])

        # biases + constants
        for p in range(NPAIR):
            nc.sync.dma_start(out=bq_sb[:, p:p + 1], in_=bq[p * P:(p + 1) * P])
            nc.sync.dma_start(out=bk_sb[:, p:p + 1], in_=bk[p * P:(p + 1) * P])
        nc.sync.dma_start(out=bv_sb[0:1, :], in_=bv[:])
        nc.gpsimd.memset(ones_f[:], 1.0)
        nc.gpsimd.memset(vS[:], 1.0)
        nc.gpsimd.partition_broadcast(bv_bc[:], bv_sb[0:1, :])

        # causal masks for the 4 diagonal offsets, stored doubled ([mask | mask])
        for di in range(4):
            d = di * 128
            for half in range(2):
                nc.gpsimd.affine_select(
                    out=mask2[:, di * 1024 + half * 512: di * 1024 + (half + 1) * 512],
                    in_=ones_f[:],
                    pattern=[[1, 512]],
                    compare_op=OP.is_ge,
                    fill=0.0,
                    base=-d,
                    channel_multiplier=-1,
                )

        # ---- staging + V projection, pipelined per tile ----
        def w_stage(w_src, wT_dst, rows, label):
            # cast one weight [rows, D] to bf16 SBUF tiles and PE-transpose
            # into wT_dst slabs; PSUM->SBUF copies on DVE.
            nt = rows // P
            for t in range(nt):
                wn = sb_nat.tile([P, w_src.shape[1]], bf16, tag="nat", name=f"wn_{label}{t}")
                nc.gpsimd.dma_start(out=wn[:], in_=w_src[t * P:(t + 1) * P, :])
                for k in range(w_src.shape[1] // P):
                    pt = ps_tr.tile([P, P], bf16, tag="ptr", name=f"pt_{label}{t}_{k}")
                    nc.tensor.transpose(pt[:], wn[:, k * P:(k + 1) * P], ident[:])
                    nc.vector.tensor_copy(
                        wT_dst[:, k * rows + t * P: k * rows + (t + 1) * P], pt[:])

        def x_stage(st):
            # cast one x s-tile to bf16 SBUF and PE-transpose into xT;
            # PSUM->SBUF copies on the (idle) ScalarE.
            xn = sb_nat.tile([P, D], bf16, tag="nat", name=f"xn{st}")
            nc.gpsimd.dma_start(out=xn[:], in_=xb[st * P:(st + 1) * P, :])
            for k in range(KT):
                pt = ps_tr.tile([P, P], bf16, tag="ptr", name=f"px{st}_{k}")
                nc.tensor.transpose(pt[:], xn[:, k * P:(k + 1) * P], ident[:])
                nc.scalar.copy(xT[:, k * S + st * P: k * S + (st + 1) * P], pt[:])

        def v_proj(st):
            vp = ps_work.tile([P, 512], f32, tag="work", name=f"vps{st}")
            for k in range(KT):
                nc.tensor.matmul(
                    vp[:],
                    xT[:, k * S + st * P: k * S + (st + 1) * P],
                    wvT[:, k * DH:(k + 1) * DH],
                    start=(k == 0), stop=(k == KT - 1))
            vdst = vS[:, st * VW:(st + 1) * VW].rearrange("p (h c) -> p h c", c=DK + 1)[:, :, 0:DK]
            nc.vector.tensor_tensor(
                vdst,
                vp[:].rearrange("p (h c) -> p h c", c=DK),
                bv_bc[:].rearrange("p (h c) -> p h c", c=DK),
                OP.add)

        w_stage(wv, wvT, DH, "v")
        for st in range(4):
            x_stage(st)
        for st in range(4):
            v_proj(st)
        w_stage(wq, wqT, DH, "q")
        for st in range(4, 8):
            x_stage(st)
        for st in range(4, 8):
            v_proj(st)
        w_stage(wk, wkT, DH, "k")
        for st in range(8, 12):
            x_stage(st)
        for st in range(8, 12):
            v_proj(st)
        w_stage(wo, woT, D, "o")
        for st in range(12, 16):
            x_stage(st)
        for st in range(12, 16):
            v_proj(st)

        # Q^T, K^T: [128 (pair dims), S] slabs; out = W_sliceT.T @ xT
        for name, wT, bias_sb, out_sb in (("q", wqT, bq_sb, qT), ("k", wkT, bk_sb, kTt)):
            for p in range(NPAIR):
                psums = [ps_work.tile([P, 512], f32, tag="work", name=f"{name}ps{p}_{_i}")
                         for _i in range(SC)]
                for k in range(KT):
                    for sc in range(SC):
                        nc.tensor.matmul(
                            psums[sc][:],
                            wT[:, k * DH + p * P: k * DH + (p + 1) * P],
                            xT[:, k * S + sc * 512: k * S + (sc + 1) * 512],
                            start=(k == 0), stop=(k == KT - 1))
                for sc in range(SC):
                    nc.vector.tensor_scalar_add(
                        out_sb[:, p * S + sc * 512: p * S + (sc + 1) * 512],
                        psums[sc][:], bias_sb[:, p:p + 1])

        # ---- attention (+ output projection interleaved per i-chunk) ----
        proj_ctx.close()
        attn_ctx = ctx.enter_context(_ES())
        ps_scores = attn_ctx.enter_context(tc.tile_pool(name="ps_scores", bufs=2, space="PSUM"))
        ps_acc = attn_ctx.enter_context(tc.tile_pool(name="ps_acc", bufs=3, space="PSUM"))
        ps_oproj = attn_ctx.enter_context(tc.tile_pool(name="ps_oproj", bufs=1, space="PSUM"))

        for ic in range(SC):
            for p in range(NPAIR):
                accA = ps_acc.tile([DK + 1, 512], f32, tag="acc", name=f"accA{ic}_{p}")
                accB = ps_acc.tile([DK + 1, 512], f32, tag="acc", name=f"accB{ic}_{p}")
                njt = 4 * ic + 4
                for jt in range(njt):
                    sps = ps_scores.tile([P, 1024], f32, tag="scores", name=f"sps{ic}{p}{jt}")
                    # scores^T for both heads of the pair, row-packed (K=64)
                    nc.tensor.matmul(
                        sps[:, 0:512],
                        kTt[0:DK, p * S + jt * P: p * S + (jt + 1) * P],
                        qT[0:DK, p * S + ic * 512: p * S + (ic + 1) * 512],
                        start=True, stop=True)
                    nc.tensor.matmul(
                        sps[:, 512:1024],
                        kTt[DK:P, p * S + jt * P: p * S + (jt + 1) * P],
                        qT[DK:P, p * S + ic * 512: p * S + (ic + 1) * 512],
                        start=True, stop=True)
                    diag = jt >= 4 * ic
                    d = (jt - 4 * ic) * P if diag else 0
                    ex = sb_exp.tile([P, 1024], bf16, tag="exp", name=f"ex{ic}{p}{jt}")
                    nc.scalar.activation(ex[:, d:1024], sps[:, d:1024], AF.Exp, scale=0.125)
                    if diag:  # causal mask on the partial diagonal tile
                        di = jt - 4 * ic
                        nc.vector.tensor_mul(
                            ex[:, d:1024], ex[:, d:1024],
                            mask2[:, di * 1024 + d:(di + 1) * 1024])
                    for hl, acc in ((0, accA), (1, accB)):
                        hv = 2 * p + hl
                        nc.tensor.matmul(
                            acc[:, d:512],
                            vS[:, jt * VW + hv * (DK + 1): jt * VW + (hv + 1) * (DK + 1)],
                            ex[:, hl * 512 + d:(hl + 1) * 512],
                            start=(jt == 0), stop=(jt == njt - 1))
                # normalize: r = 1/sumexp (fast approx on SBUF), gpsimd-broadcast
                sraw = sb_rab.tile([1, 1024], f32, tag="sraw", name=f"sr{ic}{p}")
                nc.vector.tensor_copy(sraw[0:1, 0:512], accA[DK:DK + 1, :])
                nc.vector.tensor_copy(sraw[0:1, 512:1024], accB[DK:DK + 1, :])
                rab = sb_rab.tile([1, 1024], f32, tag="rab", name=f"ra{ic}{p}")
                nc.vector.reciprocal_approx_fast(rab[0:1, :], sraw[0:1, :])
                Rs = sb_rsb.tile([P, 1024], f32, tag="rsb", name=f"rs{ic}{p}")
                nc.gpsimd.partition_broadcast(Rs[:], rab[0:1, :])
                for hl, acc in ((0, accA), (1, accB)):
                    nc.vector.tensor_mul(
                        ctxT[hl * DK:(hl + 1) * DK, p * S + ic * 512: p * S + (ic + 1) * 512],
                        acc[0:DK, :],
                        Rs[hl * DK:(hl + 1) * DK, hl * 512:(hl + 1) * 512])
            # output projection for the s-tiles of this finished i-chunk
            for st in range(4 * ic, 4 * ic + 4):
                for mc in range(2):
                    yp = ps_oproj.tile([P, 512], f32, tag="ow", name=f"yp{st}_{mc}")
                    for dt in range(NPAIR):
                        nc.tensor.matmul(
                            yp[:],
                            ctxT[:, dt * S + st * P: dt * S + (st + 1) * P],
                            woT[:, dt * D + mc * 512: dt * D + (mc + 1) * 512],
                            start=(dt == 0), stop=(dt == NPAIR - 1))
                    yt = sb_y.tile([P, 512], f32, tag="yout", name=f"yt{st}_{mc}")
                    nc.vector.tensor_copy(yt[:], yp[:])
                    nc.sync.dma_start(
                        out=y[st * P:(st + 1) * P, mc * 512:(mc + 1) * 512], in_=yt[:])

    nc.compile()
    return nc


def _get_nc():
    if "nc" not in _CACHE:
        _CACHE["nc"] = _build()
    return _CACHE["nc"]


def kernel(x, mask, Wq, bq, Wk, bk, Wv, bv, Wo, bo, **_unused):
    global LAST_EXEC_NS
    from concourse.bass_utils import run_bass_kernel_spmd

    x = np.asarray(x, dtype=np.float32)
    Wq = np.asarray(Wq, dtype=np.float32)
    Wk = np.asarray(Wk, dtype=np.float32)
    Wv = np.asarray(Wv, dtype=np.float32)
    Wo = np.asarray(Wo, dtype=np.float32)
    bq = np.asarray(bq, dtype=np.float32)
    bk = np.asarray(bk, dtype=np.float32)
    bv = np.asarray(bv, dtype=np.float32)
    bo = np.asarray(bo, dtype=np.float32)

    nc = _get_nc()
    in_maps = []
    for c in range(N_CORES):
        b, g = c // 2, c % 2
        r = slice(g * DH, (g + 1) * DH)
        in_maps.append({
            "xb": np.ascontiguousarray(x[b]),
            "wq": np.ascontiguousarray(Wq[r]),
            "wk": np.ascontiguousarray(Wk[r]),
            "wv": np.ascontiguousarray(Wv[r]),
            "wo": np.ascontiguousarray(Wo[:, r]),
            "bq": np.ascontiguousarray(bq[r]),
            "bk": np.ascontiguousarray(bk[r]),
            "bv": np.ascontiguousarray(bv[r]),
        })

    res = run_bass_kernel_spmd(nc, in_maps, list(range(N_CORES)),
                               trace=bool(os.environ.get("BASS_TRACE")))
    LAST_EXEC_NS = res.exec_time_ns

    out = np.zeros((B, S, D), dtype=np.float32)
    for c in range(N_CORES):
        out[c // 2] += res.results[c]["y"]
    out += bo[None, None, :]
    return out


# revision 22
# speedup vs baseline: 1.0373x; 1.0188x over previous
"""Multi-head attention (B=4, S=2048, D=1024, H=16, causal) on 8 TRN2 NeuronCores.

Sharding: core c -> batch b = c // 2, head-group g = c % 2 (8 heads, 512 dims).
Each core computes its heads' projections + full SxS causal attention + its
partial output projection; the host sums the two head-group partials per batch
and adds the output bias.

Per-core pipeline (all matmuls bf16 with fp32 PSUM accumulate):
  - inputs cast fp32->bf16 during DMA (SWDGE), transposed on the PE with an
    identity matmul (XBAR dma-transpose is avoided: it is slow to issue and
    concurrent use of both HWDGE rings corrupts data)
  - Q^T/K^T per head-pair slabs [128, S]; V kept natural with a ones column
    per head (ones-augmented V makes attn@V also produce sumexp rows)
  - scores^T tiles [128 j, 512 i] computed with two K=64 row-packed matmuls
    (both heads of a pair concurrently in the PE array)
  - exp on ScalarE straight out of PSUM (no max subtraction: scores are
    bounded, verified |s| <= 9.5), causal masking only on diagonal tiles via
    precomputed affine_select masks, diagonal tiles only computed from their
    first in-range column
  - softmax normalization: 1/sumexp via one reciprocal_approx_fast, then
    gpsimd partition_broadcast, then one DVE multiply per head
  - output projection y = ctx^T.T @ Wo_slice^T interleaved per i-chunk
"""

import os
import numpy as np

B, S, D = 4, 2048, 1024
H, DK = 16, 64
N_CORES = 8
DH = 512          # head dims per core (8 heads x 64)
P = 128           # partitions
KT = D // P       # 8 k-slabs
NPAIR = 4         # head pairs per core
SC = S // 512     # 4 s-chunks of 512
ST = S // P       # 16 s-tiles of 128
VW = 8 * (DK + 1)  # 520: v storage row width per s-tile (8 heads x (64 V + 1 ones))

_CACHE = {}
LAST_EXEC_NS = None


def _build():
    from contextlib import ExitStack

    import concourse.bass as bass
    import concourse.tile as tile
    from concourse import bacc, mybir
    from concourse.masks import make_identity

    f32 = mybir.dt.float32
    bf16 = mybir.dt.bfloat16
    AF = mybir.ActivationFunctionType
    OP = mybir.AluOpType

    nc = bacc.Bacc("TRN2", target_bir_lowering=False, debug=False,
                   num_devices=N_CORES)

    xb = nc.dram_tensor("xb", [S, D], f32, kind="ExternalInput").ap()
    wq = nc.dram_tensor("wq", [DH, D], f32, kind="ExternalInput").ap()
    wk = nc.dram_tensor("wk", [DH, D], f32, kind="ExternalInput").ap()
    wv = nc.dram_tensor("wv", [DH, D], f32, kind="ExternalInput").ap()
    wo = nc.dram_tensor("wo", [D, DH], f32, kind="ExternalInput").ap()
    bq = nc.dram_tensor("bq", [DH], f32, kind="ExternalInput").ap()
    bk = nc.dram_tensor("bk", [DH], f32, kind="ExternalInput").ap()
    bv = nc.dram_tensor("bv", [DH], f32, kind="ExternalInput").ap()
    y = nc.dram_tensor("y", [S, D], f32, kind="ExternalOutput").ap()

    with tile.TileContext(nc) as tc, ExitStack() as ctx:
        from contextlib import ExitStack as _ES

        persist = ctx.enter_context(tc.tile_pool(name="persist", bufs=1))

        # persistent SBUF tensors
        xT = persist.tile([P, KT * S], bf16, tag="xT")            # k-slab k at cols [k*S, (k+1)*S)
        wqT = persist.tile([P, KT * DH], bf16, tag="wqT")         # [128k, 512dq] per slab
        wkT = persist.tile([P, KT * DH], bf16, tag="wkT")
        wvT = persist.tile([P, KT * DH], bf16, tag="wvT")
        woT = persist.tile([P, NPAIR * D], bf16, tag="woT")       # d-slab dt at cols [dt*D, ...)
        qT = persist.tile([P, NPAIR * S], bf16, tag="qT")         # pair p at cols [p*S, ...)
        kTt = persist.tile([P, NPAIR * S], bf16, tag="kTt")
        vS = persist.tile([P, ST * VW], bf16, tag="vS")           # s-tile jt at cols [jt*VW, ...)
        ctxT = persist.tile([P, NPAIR * S], bf16, tag="ctxT")
        mask2 = persist.tile([P, 4 * 1024], bf16, tag="mask2")    # 4 diagonal masks, doubled
        bq_sb = persist.tile([P, NPAIR], f32, tag="bq_sb")
        bk_sb = persist.tile([P, NPAIR], f32, tag="bk_sb")
        bv_sb = persist.tile([1, DH], f32, tag="bv_sb")
        bv_bc = persist.tile([P, DH], f32, tag="bv_bc")
        ones_f = persist.tile([P, 512], f32, tag="ones_f")
        ident = persist.tile([P, P], bf16, tag="ident")

        proj_ctx = ctx.enter_context(_ES())
        ps_work = proj_ctx.enter_context(tc.tile_pool(name="ps_work", bufs=4, space="PSUM"))
        ps_tr = proj_ctx.enter_context(tc.tile_pool(name="ps_tr", bufs=4, space="PSUM"))
        sb_nat = proj_ctx.enter_context(tc.tile_pool(name="sb_nat", bufs=6))

        make_identity(nc, ident[:])

        # biases + constants
        for p in range(NPAIR):
            nc.sync.dma_start(out=bq_sb[:, p:p + 1], in_=bq[p * P:(p + 1) * P])
            nc.sync.dma_start(out=bk_sb[:, p:p + 1], in_=bk[p * P:(p + 1) * P])
        nc.sync.dma_start(out=bv_sb[0:1, :], in_=bv[:])
        nc.gpsimd.memset(ones_f[:], 1.0)
        nc.gpsimd.memset(vS[:], 1.0)
        nc.gpsimd.partition_broadcast(bv_bc[:], bv_sb[0:1, :])

        # causal masks for the 4 diagonal offsets, stored doubled ([mask | mask])
        for di in range(4):
            d = di * 128
            for half in range(2):
                nc.gpsimd.affine_select(
                    out=mask2[:, di * 1024 + half * 512: di * 1024 + (half + 1) * 512],
                    in_=ones_f[:],
                    pattern=[[1, 512]],
                    compare_op=OP.is_ge,
                    fill=0.0,
                    base=-d,
                    channel_multiplier=-1,
                )

        # ---- staging + V projection, pipelined per tile ----
        def w_stage(w_src, wT_dst, rows, label):
            # cast one weight [rows, D] to bf16 SBUF tiles and PE-transpose
            # into wT_dst slabs; PSUM->SBUF copies on DVE.
            nt = rows // P
            for t in range(nt):
                wn = sb_nat.tile([P, w_src.shape[1]], bf16, tag="nat", name=f"wn_{label}{t}")
                nc.gpsimd.dma_start(out=wn[:], in_=w_src[t * P:(t + 1) * P, :])
                for k in range(w_src.shape[1] // P):
                    pt = ps_tr.tile([P, P], bf16, tag="ptr", name=f"pt_{label}{t}_{k}")
                    nc.tensor.transpose(pt[:], wn[:, k * P:(k + 1) * P], ident[:])
                    nc.vector.tensor_copy(
                        wT_dst[:, k * rows + t * P: k * rows + (t + 1) * P], pt[:])

        def x_stage(st):
            # cast one x s-tile to bf16 SBUF and PE-transpose into xT;
            # PSUM->SBUF copies on the (idle) ScalarE.
            xn = sb_nat.tile([P, D], bf16, tag="nat", name=f"xn{st}")
            nc.gpsimd.dma_start(out=xn[:], in_=xb[st * P:(st + 1) * P, :])
            for k in range(KT):
                pt = ps_tr.tile([P, P], bf16, tag="ptr", name=f"px{st}_{k}")
                nc.tensor.transpose(pt[:], xn[:, k * P:(k + 1) * P], ident[:])
                nc.scalar.copy(xT[:, k * S + st * P: k * S + (st + 1) * P], pt[:])

        def v_proj(st):
            vp = ps_work.tile([P, 512], f32, tag="work", name=f"vps{st}")
            for k in range(KT):
                nc.tensor.matmul(
                    vp[:],
                    xT[:, k * S + st * P: k * S + (st + 1) * P],
                    wvT[:, k * DH:(k + 1) * DH],
                    start=(k == 0), stop=(k == KT - 1))
            vdst = vS[:, st * VW:(st + 1) * VW].rearrange("p (h c) -> p h c", c=DK + 1)[:, :, 0:DK]
            nc.vector.tensor_tensor(
                vdst,
                vp[:].rearrange("p (h c) -> p h c", c=DK),
                bv_bc[:].rearrange("p (h c) -> p h c", c=DK),
                OP.add)

        w_stage(wv, wvT, DH, "v")
        for st in range(4):
            x_stage(st)
        for st in range(4):
            v_proj(st)
        w_stage(wq, wqT, DH, "q")
        for st in range(4, 8):
            x_stage(st)
        for st in range(4, 8):
            v_proj(st)
        w_stage(wk, wkT, DH, "k")
        for st in range(8, 12):
            x_stage(st)
        for st in range(8, 12):
            v_proj(st)
        w_stage(wo, woT, D, "o")
        for st in range(12, 16):
            x_stage(st)
        for st in range(12, 16):
            v_proj(st)

        # Q^T, K^T: [128 (pair dims), S] slabs; out = W_sliceT.T @ xT
        for name, wT, bias_sb, out_sb in (("q", wqT, bq_sb, qT), ("k", wkT, bk_sb, kTt)):
            for p in range(NPAIR):
                psums = [ps_work.tile([P, 512], f32, tag="work", name=f"{name}ps{p}_{_i}")
                         for _i in range(SC)]
                for k in range(KT):
                    for sc in range(SC):
                        nc.tensor.matmul(
                            psums[sc][:],
                            wT[:, k * DH + p * P: k * DH + (p + 1) * P],
                            xT[:, k * S + sc * 512: k * S + (sc + 1) * 512],
                            start=(k == 0), stop=(k == KT - 1))
                for sc in range(SC):
                    nc.vector.tensor_scalar_add(
                        out_sb[:, p * S + sc * 512: p * S + (sc + 1) * 512],
                        psums[sc][:], bias_sb[:, p:p + 1])

        # ---- attention (+ output projection interleaved per i-chunk) ----
        proj_ctx.close()
        attn_ctx = ctx.enter_context(_ES())
        ps_scores = attn_ctx.enter_context(tc.tile_pool(name="ps_scores", bufs=2, space="PSUM"))
        ps_acc = attn_ctx.enter_context(tc.tile_pool(name="ps_acc", bufs=3, space="PSUM"))
        ps_oproj = attn_ctx.enter_context(tc.tile_pool(name="ps_oproj", bufs=1, space="PSUM"))
        sb_exp = attn_ctx.enter_context(tc.tile_pool(name="sb_exp", bufs=9))
        sb_y = attn_ctx.enter_context(tc.tile_pool(name="sb_y", bufs=3))
        sb_rab = attn_ctx.enter_context(tc.tile_pool(name="sb_rab", bufs=2))
        sb_rsb = attn_ctx.enter_context(tc.tile_pool(name="sb_rsb", bufs=2))

        for ic in range(SC):
            for p in range(NPAIR):
                accA = ps_acc.tile([DK + 1, 512], f32, tag="acc", name=f"accA{ic}_{p}")
                accB = ps_acc.tile([DK + 1, 512], f32, tag="acc", name=f"accB{ic}_{p}")
                njt = 4 * ic + 4
                for jt in range(njt):
                    sps = ps_scores.tile([P, 1024], f32, tag="scores", name=f"sps{ic}{p}{jt}")
                    # scores^T for both heads of the pair, row-packed (K=64)
                    nc.tensor.matmul(
                        sps[:, 0:512],
                        kTt[0:DK, p * S + jt * P: p * S + (jt + 1) * P],
                        qT[0:DK, p * S + ic * 512: p * S + (ic + 1) * 512],
                        start=True, stop=True)
                    nc.tensor.matmul(
                        sps[:, 512:1024],
                        kTt[DK:P, p * S + jt * P: p * S + (jt + 1) * P],
                        qT[DK:P, p * S + ic * 512: p * S + (ic + 1) * 512],
                        start=True, stop=True)
                    diag = jt >= 4 * ic
                    d = (jt - 4 * ic) * P if diag else 0
                    ex = sb_exp.tile([P, 1024], bf16, tag="exp", name=f"ex{ic}{p}{jt}")
                    nc.scalar.activation(ex[:, d:1024], sps[:, d:1024], AF.Exp, scale=0.125)
                    if diag:  # causal mask on the partial diagonal tile
                        di = jt - 4 * ic
                        nc.vector.tensor_mul(
                            ex[:, d:1024], ex[:, d:1024],
                            mask2[:, di * 1024 + d:(di + 1) * 1024])
                    for hl, acc in ((0, accA), (1, accB)):
                        hv = 2 * p + hl
                        nc.tensor.matmul(
                            acc[:, d:512],
                            vS[:, jt * VW + hv * (DK + 1): jt * VW + (hv + 1) * (DK + 1)],
                            ex[:, hl * 512 + d:(hl + 1) * 512],
                            start=(jt == 0), stop=(jt == njt - 1))
                # normalize: r = 1/sumexp (fast approx on SBUF), gpsimd-broadcast
                sraw = sb_rab.tile([1, 1024], f32, tag="sraw", name=f"sr{ic}{p}")
                nc.vector.tensor_copy(sraw[0:1, 0:512], accA[DK:DK + 1, :])
                nc.vector.tensor_copy(sraw[0:1, 512:1024], accB[DK:DK + 1, :])
                rab = sb_rab.tile([1, 1024], f32, tag="rab", name=f"ra{ic}{p}")
                nc.vector.reciprocal_approx_fast(rab[0:1, :], sraw[0:1, :])
                Rs = sb_rsb.tile([P, 1024], f32, tag="rsb", name=f"rs{ic}{p}")
                nc.gpsimd.partition_broadcast(Rs[:], rab[0:1, :])
                for hl, acc in ((0, accA), (1, accB)):
                    nc.vector.tensor_mul(
                        ctxT[hl * DK:(hl + 1) * DK, p * S + ic * 512: p * S + (ic + 1) * 512],
                        acc[0:DK, :],
                        Rs[hl * DK:(hl + 1) * DK, hl * 512:(hl + 1) * 512])
            # output projection for the s-tiles of this finished i-chunk
            for st in range(4 * ic, 4 * ic + 4):
                for mc in range(2):
                    yp = ps_oproj.tile([P, 512], f32, tag="ow", name=f"yp{st}_{mc}")
                    for dt in range(NPAIR):
                        nc.tensor.matmul(
                            yp[:],
                            ctxT[:, dt * S + st * P: dt * S + (st + 1) * P],
                            woT[:, dt * D + mc * 512: dt * D + (mc + 1) * 512],
                            start=(dt == 0), stop=(dt == NPAIR - 1))
                    yt = sb_y.tile([P, 512], f32, tag="yout", name=f"yt{st}_{mc}")
                    nc.vector.tensor_copy(yt[:], yp[:])
                    nc.sync.dma_start(
                        out=y[st * P:(st + 1) * P, mc * 512:(mc + 1) * 512], in_=yt[:])

    nc.compile()
    return nc


def _get_nc():
    if "nc" not in _CACHE:
        _CACHE["nc"] = _build()
    return _CACHE["nc"]


def kernel(x, mask, Wq, bq, Wk, bk, Wv, bv, Wo, bo, **_unused):
    global LAST_EXEC_NS
    from concourse.bass_utils import run_bass_kernel_spmd

    x = np.asarray(x, dtype=np.float32)
    Wq = np.asarray(Wq, dtype=np.float32)
    Wk = np.asarray(Wk, dtype=np.float32)
    Wv = np.asarray(Wv, dtype=np.float32)
    Wo = np.asarray(Wo, dtype=np.float32)
    bq = np.asarray(bq, dtype=np.float32)
    bk = np.asarray(bk, dtype=np.float32)
    bv = np.asarray(bv, dtype=np.float32)
    bo = np.asarray(bo, dtype=np.float32)

    nc = _get_nc()
    in_maps = []
    for c in range(N_CORES):
        b, g = c // 2, c % 2
        r = slice(g * DH, (g + 1) * DH)
        in_maps.append({
            "xb": np.ascontiguousarray(x[b]),
            "wq": np.ascontiguousarray(Wq[r]),
            "wk": np.ascontiguousarray(Wk[r]),
            "wv": np.ascontiguousarray(Wv[r]),
            "wo": np.ascontiguousarray(Wo[:, r]),
            "bq": np.ascontiguousarray(bq[r]),
            "bk": np.ascontiguousarray(bk[r]),
            "bv": np.ascontiguousarray(bv[r]),
        })

    res = run_bass_kernel_spmd(nc, in_maps, list(range(N_CORES)),
                               trace=bool(os.environ.get("BASS_TRACE")))
    LAST_EXEC_NS = res.exec_time_ns

    out = np.zeros((B, S, D), dtype=np.float32)
    for c in range(N_CORES):
        out[c // 2] += res.results[c]["y"]
    out += bo[None, None, :]
    return out
